# revision 1
# baseline (speedup 1.0000x reference)
"""Batched GCN (5-layer message passing) on 8 Trainium2 NeuronCores.

Problem: nn_BatchedGNNModel_45191645888927
  x [1024, 192, 6], inputs [1024, 192, 3], adjacency [1024, 192, 192]
  (identical per batch element), 5 GCN layers (leaky_relu 0.2 on 1-4).

Strategy (pure data parallel, 128 batch elements per core):
  * adjacency is identical across batch -> normalized operator An and its
    degree-scaled variants are built once on host (192x192) and shipped to
    every core; the 151MB adjacency tensor is never touched on device.
  * leaky_relu positive homogeneity: lrelu(diag(s) v) = diag(s) lrelu(v)
    (s = An row sums > 0) lets us carry "unscaled" activations so the GCN
    bias becomes per-feature (1 (x) b) -> fused into ONE scalar-engine
    activation pass (Lrelu, per-partition bias) in feature-major layout.
        hhat_k = lrelu(M (hhat_{k-1} W_k^T) + 1 (x) b_k)
    with M = diag(1/s) An (layer 1), diag(1/s) An diag(s) (layers 2-4),
    and out = An diag(s) hhat_4 W5^T + s (x) b5 (b5 term added on host).
  * per layer on device (fp16 operands, fp32 PSUM accumulation), default
    tmode="iv2" (no transposes anywhere -- DMA XBAR transposes measured as a
    serialized ~2.5us/op resource and PE transposes burn tensor cycles):
      W-mult : stationary = per-batch feature-major activation chunks
               (nodes 0:128 / 128:192 as lhsT columns), moving = W^T
               -> PSUM already NODE-major [i, o] (the "transpose" is free)
      evac   : DVE/ACT copy PSUM -> SBUF fp16 (tn / tnb node tiles)
      An-mult: stationary = tn/tnb, moving = M^T chunks -> G feature-major
      act    : ACT Lrelu(G + b) with per-partition bias, PSUM -> SBUF fp16
    with the An-mult of pair p emitted after the W-mult of pair p+1
    (software pipelining so the PE never waits on evacuations).
  * layer 5: same W-mult form at N=3 packing all batches into one PSUM
    bank, then one batched An matmul at N=3*batch; row clamping, the
    (zero) b5 term and layout transposition happen on host.
  * TRN2 allows only 1 sync wait per instruction (the walrus XPOSE/DVE
    descriptors reject more); bass_rust.generate_event_semaphores splits
    Tile's multi-wait instructions after scheduling.
"""

import contextlib
import os
import numpy as np

import bass_rust
import concourse.bass as bass
import concourse.mybir as mybir
import concourse.tile as tile
from concourse.bass_utils import run_bass_kernel_spmd

FP16 = mybir.dt.float16
FP32 = mybir.dt.float32

B = 1024
NCORES = 8
BC = B // NCORES          # 128 batch elements per core
NBLK = 32                 # batch block per pipeline stage
N = 192                   # nodes
H = 128                   # hidden
FIN = 6
FOUT = 3
CLAMP_ROWS = [0, 63, 127, 191]

_CACHE = {}


def _build(act="lrelu", repeat=1, skip=(), bufs=None, an2="both", tmode="iv2", tpeng="act", nblk=NBLK):
    """Build the Bass/Tile program once. act: 'lrelu' | 'relu' (relu only
    for CoreSim structural debugging, which cannot interpret Lrelu)."""
    _bufs = dict(pz=3, pg=4, tp=3, ts=4, tn=6, y=2, zb=1)
    NB = nblk
    _bufs.update(bufs or {})
    bufs = _bufs
    skip = set(skip)
    nc = bass.Bass("TRN2", target_bir_lowering=False, debug=False)

    # ---- DRAM parameters (per-core) ----
    d_x = nc.dram_tensor("xfeat", [FIN, BC, N], FP16, kind="ExternalInput").ap()
    pad_b = tmode not in ("pe", "iv", "iv2", "iv3")
    bsh = [128, N] if pad_b else [64, N]
    bnm = "" if pad_b else "u"
    d_m1a = nc.dram_tensor("m1T_a", [128, N], FP16, kind="ExternalInput").ap()
    d_m1b = nc.dram_tensor("m1T_b" + bnm, bsh, FP16, kind="ExternalInput").ap()
    d_mma = nc.dram_tensor("mmT_a", [128, N], FP16, kind="ExternalInput").ap()
    d_mmb = nc.dram_tensor("mmT_b" + bnm, bsh, FP16, kind="ExternalInput").ap()
    d_m5a = nc.dram_tensor("m5T_a", [128, N], FP16, kind="ExternalInput").ap()
    d_m5b = nc.dram_tensor("m5T_b", [64, N], FP16, kind="ExternalInput").ap()
    d_w1 = nc.dram_tensor("w1T", [FIN, H], FP16, kind="ExternalInput").ap()
    d_w2 = nc.dram_tensor("w2T", [H, H], FP16, kind="ExternalInput").ap()
    d_w3 = nc.dram_tensor("w3T", [H, H], FP16, kind="ExternalInput").ap()
    d_w4 = nc.dram_tensor("w4T", [H, H], FP16, kind="ExternalInput").ap()
    d_w5 = nc.dram_tensor("w5T", [H, FOUT], FP16, kind="ExternalInput").ap()
    d_b = [
        nc.dram_tensor(f"b{k}", [H, 1], FP32, kind="ExternalInput").ap()
        for k in (1, 2, 3, 4)
    ]
    d_ident = nc.dram_tensor("ident", [128, 128], FP16, kind="ExternalInput").ap()
    d_olo = nc.dram_tensor("out_lo", [128, BC, FOUT], FP32, kind="ExternalOutput").ap()
    d_ohi = nc.dram_tensor("out_hi", [64, BC, FOUT], FP32, kind="ExternalOutput").ap()

    afun = (
        mybir.ActivationFunctionType.Lrelu
        if act == "lrelu"
        else mybir.ActivationFunctionType.Relu
    )

    with tile.TileContext(nc) as tc:
        with (
            tc.tile_pool(name="const", bufs=1) as cpool,
            tc.tile_pool(name="ypool", bufs=bufs["y"]) as ypool,
            tc.tile_pool(name="tspool", bufs=bufs["ts"]) as tspool,
            tc.tile_pool(name="tnpool", bufs=bufs["tn"]) as tnpool,
            tc.tile_pool(name="opool", bufs=2) as opool,
            tc.tile_pool(name="pz", bufs=bufs["pz"], space="PSUM") as pz,
            tc.tile_pool(name="pg", bufs=bufs["pg"], space="PSUM") as pg,
            tc.tile_pool(name="ptp", bufs=bufs["tp"], space="PSUM") as ptp,
            tc.tile_pool(name="pzb", bufs=bufs["zb"], space="PSUM") as pzb,
        ):
            # ---- load constants ----
            xf = cpool.tile([FIN, BC, N], FP16, tag="xf")
            nc.sync.dma_start(xf[:], d_x)
            m1a = cpool.tile([128, N], FP16, tag="m1a")
            m1b = cpool.tile(bsh, FP16, tag="m1b")
            mma = cpool.tile([128, N], FP16, tag="mma")
            mmb = cpool.tile(bsh, FP16, tag="mmb")
            m5a = cpool.tile([128, N], FP16, tag="m5a")
            m5b = cpool.tile([64, N], FP16, tag="m5b")
            nc.sync.dma_start(m1a[:], d_m1a)
            nc.sync.dma_start(m1b[:], d_m1b)
            nc.sync.dma_start(mma[:], d_mma)
            nc.sync.dma_start(mmb[:], d_mmb)
            nc.sync.dma_start(m5a[:], d_m5a)
            nc.sync.dma_start(m5b[:], d_m5b)
            w1 = cpool.tile([FIN, H], FP16, tag="w1")
            w2 = cpool.tile([H, H], FP16, tag="w2")
            w3 = cpool.tile([H, H], FP16, tag="w3")
            w4 = cpool.tile([H, H], FP16, tag="w4")
            w5 = cpool.tile([H, FOUT], FP16, tag="w5")
            nc.sync.dma_start(w1[:], d_w1)
            nc.sync.dma_start(w2[:], d_w2)
            nc.sync.dma_start(w3[:], d_w3)
            nc.sync.dma_start(w4[:], d_w4)
            nc.sync.dma_start(w5[:], d_w5)
            ident = cpool.tile([128, 128], FP16, tag="ident")
            nc.sync.dma_start(ident[:], d_ident)
            bt = []
            for k in range(4):
                b_ = cpool.tile([H, 1], FP32, tag=f"b{k}")
                nc.sync.dma_start(b_[:], d_b[k])
                bt.append(b_)

            wk = [w1, w2, w3, w4]
            n_pairs = NB // 2

            rep_cm = tc.For_i(0, repeat, 1) if repeat > 1 else contextlib.nullcontext()
            with rep_cm:
              for blk in range(BC // NB):
                  b0 = blk * NB
                  y_prev = None  # feature-major activations [H, NB, N] fp16
                  for k in range(4):
                      mt_a, mt_b = (m1a, m1b) if k == 0 else (mma, mmb)
                      y_cur = ypool.tile([H, NB, N], FP16, tag="y")
                      if tmode in ("iv", "iv2", "iv3"):
                          fin = FIN if k == 0 else H
                          depth = 2 if tmode == "iv3" else 1
                          pending = []  # (tn, tnb, p) awaiting An+act
                          def an_and_act(tn, tnb, p):
                              g = pg.tile([H, 2, N], FP32, tag="g")
                              for e in range(2):
                                  nc.tensor.matmul(
                                      g[:, e, :], tn[:, e, :], mt_a[:],
                                      start=True, stop=False,
                                  )
                                  nc.tensor.matmul(
                                      g[:, e, :], tnb[:, e, :], mt_b[:],
                                      start=False, stop=True,
                                  )
                              nc.scalar.activation(
                                  y_cur[:, 2 * p : 2 * p + 2, :],
                                  g[:],
                                  afun,
                                  bias=bt[k][:],
                                  scale=1.0,
                                  alpha=0.2,
                              )
                          for p in range(n_pairs):
                              # W-mult directly to node-major (stationary =
                              # per-batch activation chunks, moving = W^T)
                              za = pz.tile([128, 2, H], FP32, tag="z")
                              zb = pzb.tile([64, 2, H], FP32, tag="zb")
                              for e in range(2):
                                  if k == 0:
                                      ysrc = xf[:, b0 + 2 * p + e, :]
                                  else:
                                      ysrc = y_prev[:, 2 * p + e, :]
                                  nc.tensor.matmul(
                                      za[:, e, :], ysrc[:, 0:128], wk[k][:],
                                      start=True, stop=True,
                                  )
                                  nc.tensor.matmul(
                                      zb[:, e, :], ysrc[:, 128:192], wk[k][:],
                                      start=True, stop=True,
                                  )
                              tn = tnpool.tile([128, 2, H], FP16, tag="tn")
                              tnb = tnpool.tile([64, 2, H], FP16, tag="tnb")
                              nc.vector.tensor_copy(tn[:], za[:])
                              nc.scalar.copy(tnb[:], zb[:])
                              if tmode in ("iv2", "iv3"):
                                  # software-pipeline: An of pair p-depth
                                  # lands after W of pair p, hiding evac
                                  # latency from the PE stream
                                  pending.append((tn, tnb, p))
                                  if len(pending) > depth:
                                      an_and_act(*pending.pop(0))
                              else:
                                  an_and_act(tn, tnb, p)
                          for args in pending:
                              an_and_act(*args)
                          y_prev = y_cur
                          continue
                      for p in range(n_pairs):
                          # ---- W-mult (stationary W^T, moving feat-major) ----
                          z = pz.tile([H, 2, N], FP32, tag="z")
                          if k == 0:
                              rhs = xf[:, b0 + 2 * p : b0 + 2 * p + 2, :]
                          else:
                              rhs = y_prev[:, 2 * p : 2 * p + 2, :]
                          nc.tensor.matmul(z[:], wk[k][:], rhs, start=True, stop=True)
                          # evac PSUM -> SBUF fp16 (both batches in one op)
                          ts = tspool.tile([H, 2, N], FP16, tag="ts")
                          nc.vector.tensor_copy(ts[:], z[:])
                          g = pg.tile([H, 2, N], FP32, tag="g")
                          for e in range(2):
                              # ---- transpose feat->node (DMA XBAR, fp16) ----
                              # tn[:,0] partitions = nodes 0:128
                              # tn[:,1] partitions = nodes 64:192 (overlap trick)
                              tn = tnpool.tile([128, 2, H], FP16, tag="tn")
                              if "transpose" in skip:
                                  nc.vector.memset(tn[:, :, 0:2], 0.0)
                              elif tmode == "dma":
                                  nc.sync.dma_start(
                                      out=tn[:, 0, :], in_=ts[:, e, 0:128], transpose=True
                                  )
                                  nc.sync.dma_start(
                                      out=tn[:, 1, :], in_=ts[:, e, 64:192], transpose=True
                                  )
                              else:
                                  # PE transposes (fp16, 1 cyc/row) + DVE evac
                                  tp1 = ptp.tile([128, 128], FP16, tag="tp")
                                  nc.tensor.transpose(
                                      tp1[:], ts[:, e, 0:128], ident[:]
                                  )
                                  tp2 = ptp.tile([64, 128], FP16, tag="tp")
                                  nc.tensor.transpose(
                                      tp2[:], ts[:, e, 128:192], ident[:]
                                  )
                                  if tpeng == "act":
                                      nc.scalar.copy(tn[:, 0, :], tp1[:])
                                      nc.scalar.copy(tn[0:64, 1, :], tp2[:])
                                  else:
                                      nc.vector.tensor_copy(tn[:, 0, :], tp1[:])
                                      nc.vector.tensor_copy(tn[0:64, 1, :], tp2[:])
                              # ---- An-mult (stationary nodes, moving M^T) ----
                              if "anmult" in skip or an2 == "one":
                                  nc.tensor.matmul(
                                      g[:, e, :], tn[:, 0, :], mt_a[:],
                                      start=True, stop=True,
                                  )
                              elif an2 == "dup0":
                                  nc.tensor.matmul(
                                      g[:, e, :], tn[:, 0, :], mt_a[:],
                                      start=True, stop=False,
                                  )
                                  nc.tensor.matmul(
                                      g[:, e, :], tn[:, 0, :], mt_a[:],
                                      start=False, stop=True,
                                  )
                              elif an2 == "base0":
                                  nc.tensor.matmul(
                                      g[:, e, :], tn[:, 0, :], mt_a[:],
                                      start=True, stop=False,
                                  )
                                  nc.tensor.matmul(
                                      g[:, e, :],
                                      tn[0:64, 1, :],
                                      mt_b[0:64, :],
                                      start=False,
                                      stop=True,
                                  )
                              elif tmode == "pe":
                                  nc.tensor.matmul(
                                      g[:, e, :],
                                      tn[:, 0, :],
                                      mt_a[:],
                                      start=True,
                                      stop=False,
                                  )
                                  nc.tensor.matmul(
                                      g[:, e, :],
                                      tn[0:64, 1, :],
                                      mt_b[0:64, :],
                                      start=False,
                                      stop=True,
                                  )
                              else:
                                  nc.tensor.matmul(
                                      g[:, e, :],
                                      tn[:, 0, :],
                                      mt_a[:],
                                      start=True,
                                      stop=False,
                                  )
                                  nc.tensor.matmul(
                                      g[:, e, :],
                                      tn[64:128, 1, :],
                                      mt_b[64:128, :],
                                      start=False,
                                      stop=True,
                                  )
                          nc.scalar.activation(
                              y_cur[:, 2 * p : 2 * p + 2, :],
                              g[:],
                              afun,
                              bias=bt[k][:],
                              scale=1.0,
                              alpha=0.2,
                          )
                      y_prev = y_cur

                  # ---- layer 5 ----
                  # z5 = hhat4 W5^T, node-major, all NB batches packed in PSUM
                  z5a = pz.tile([128, NB, FOUT], FP32, tag="z")
                  z5b = pz.tile([64, NB, FOUT], FP32, tag="z")
                  for bb in range(NB):
                      nc.tensor.matmul(
                          z5a[:, bb, :], y_prev[:, bb, 0:128], w5[:], start=True, stop=True
                      )
                      nc.tensor.matmul(
                          z5b[:, bb, :], y_prev[:, bb, 128:192], w5[:], start=True, stop=True
                      )
                  z5sa = opool.tile([128, NB, FOUT], FP16, tag="z5sa")
                  z5sb = opool.tile([64, NB, FOUT], FP16, tag="z5sb")
                  nc.vector.tensor_copy(z5sa[:], z5a[:])
                  nc.vector.tensor_copy(z5sb[:], z5b[:])
                  # h5 = M5 z5 : out [i', (b, o)]
                  h5a = pg.tile([128, NB, FOUT], FP32, tag="g")
                  h5b = pg.tile([64, NB, FOUT], FP32, tag="g")
                  nc.tensor.matmul(h5a[:], m5a[:, 0:128], z5sa[:], start=True, stop=False)
                  nc.tensor.matmul(h5a[:], m5b[:, 0:128], z5sb[:], start=False, stop=True)
                  nc.tensor.matmul(h5b[:], m5a[:, 128:192], z5sa[:], start=True, stop=False)
                  nc.tensor.matmul(h5b[:], m5b[:, 128:192], z5sb[:], start=False, stop=True)
                  oa = opool.tile([128, NB, FOUT], FP32, tag="oa")
                  ob = opool.tile([64, NB, FOUT], FP32, tag="ob")
                  nc.scalar.copy(oa[:], h5a[:])
                  nc.scalar.copy(ob[:], h5b[:])
                  nc.sync.dma_start(d_olo[:, b0 : b0 + NB, :], oa[:])
                  nc.sync.dma_start(d_ohi[:, b0 : b0 + NB, :], ob[:])

    # Normalize to <=1 sync wait per instruction (TRN2 HW limit; the DMA
    # XPOSE descriptor in particular rejects more) by splitting excess waits
    # onto InstEventSemaphore instructions, exactly like bacc does.
    bass_rust.generate_event_semaphores(nc)
    return nc


def _get_nc(act="lrelu", repeat=1, skip=(), bufs=None, an2="both", tmode="iv2",
            tpeng="act", nblk=NBLK):
    key = (act, repeat, tuple(sorted(skip)),
           tuple(sorted((bufs or {}).items())), an2, tmode, tpeng, nblk)
    if key not in _CACHE:
        _CACHE[key] = _build(act, repeat, skip, bufs, an2, tmode, tpeng, nblk)
    return _CACHE[key]


def _host_prep(x, inputs, adjacency, W1, b1, W2, b2, W3, b3, W4, b4, W5, b5):
    """Build per-core input maps + host-side constants."""
    x = np.asarray(x, np.float32)
    inputs = np.asarray(inputs, np.float32)
    A = np.asarray(adjacency[0], np.float32)  # identical across batch

    deg = A.sum(-1)
    dinv = np.where(deg == 0.0, 0.0, deg**-0.5)
    An = A * dinv[:, None] * dinv[None, :]
    s = An.sum(-1)  # row sums (all > 0 for this graph)
    M1 = An / s[:, None]
    Mm = An * (s[None, :] / s[:, None])
    M5 = An * s[None, :]

    def chunks(M, pad_b=False):
        T = np.ascontiguousarray(M.T.astype(np.float16))  # [j, i']
        lo, hi = T[0:128], T[128:192]
        if pad_b:
            # padded so the sbuf tile slice [64:128] aligns with lhsT base
            # partition 64 (tn2[64:128] holds nodes 128:192)
            p = np.zeros((128, N), np.float16)
            p[64:128] = hi
            hi = p
        return lo, np.ascontiguousarray(hi)

    m1a, m1b = chunks(M1, pad_b=True)
    mma, mmb = chunks(Mm, pad_b=True)
    m1au, m1bu = chunks(M1)
    mmau, mmbu = chunks(Mm)
    m5a, m5b = chunks(M5)

    # clamp boundary vertices into x (features 0:3 only)
    x_cl = x.copy()
    x_cl[:, CLAMP_ROWS, 0:3] = inputs[:, CLAMP_ROWS, :]
    xfeat = np.ascontiguousarray(x_cl.transpose(2, 0, 1).astype(np.float16))  # [6,B,N]

    consts = dict(
        m1T_a=m1a, m1T_b=m1b, mmT_a=mma, mmT_b=mmb, m5T_a=m5a, m5T_b=m5b,
        m1T_bu=m1bu, mmT_bu=mmbu,
        w1T=np.ascontiguousarray(np.asarray(W1, np.float32).T.astype(np.float16)),
        w2T=np.ascontiguousarray(np.asarray(W2, np.float32).T.astype(np.float16)),
        w3T=np.ascontiguousarray(np.asarray(W3, np.float32).T.astype(np.float16)),
        w4T=np.ascontiguousarray(np.asarray(W4, np.float32).T.astype(np.float16)),
        w5T=np.ascontiguousarray(np.asarray(W5, np.float32).T.astype(np.float16)),
        b1=np.asarray(b1, np.float32).reshape(H, 1),
        b2=np.asarray(b2, np.float32).reshape(H, 1),
        b3=np.asarray(b3, np.float32).reshape(H, 1),
        b4=np.asarray(b4, np.float32).reshape(H, 1),
        ident=np.eye(128, dtype=np.float16),
    )
    in_maps = []
    for c in range(NCORES):
        m = dict(consts)
        m["xfeat"] = np.ascontiguousarray(xfeat[:, c * BC : (c + 1) * BC, :])
        in_maps.append(m)
    return in_maps, s


def _assemble(results, s, b5, inputs):
    """results: per-core dicts with out_lo [128,BC,3], out_hi [64,BC,3]."""
    outs = []
    for c in range(NCORES):
        lo = np.asarray(results[c]["out_lo"], np.float32)
        hi = np.asarray(results[c]["out_hi"], np.float32)
        full = np.concatenate([lo, hi], axis=0)  # [192, BC, 3]
        outs.append(full.transpose(1, 0, 2))  # [BC, 192, 3]
    out = np.concatenate(outs, axis=0)  # [B, 192, 3]
    b5 = np.asarray(b5, np.float32)
    if np.any(b5 != 0.0):
        out = out + np.asarray(s, np.float32)[None, :, None] * b5[None, None, :]
    out[:, CLAMP_ROWS, :] = np.asarray(inputs, np.float32)[:, CLAMP_ROWS, :]
    return out


def kernel(**inputs):
    nc = _get_nc(os.environ.get("GNN_ACT", "lrelu"))
    in_maps, s = _host_prep(**inputs)
    res = run_bass_kernel_spmd(nc, in_maps, list(range(NCORES)))
    return _assemble(res.results, s, inputs["b5"], inputs["inputs"])


if __name__ == "__main__":
    nc = _get_nc()
    print("built ok")

